# revision 7
# baseline (speedup 1.0000x reference)
"""Adaptive embedding (nn_AdaptiveEmbedding) Trainium2 Bass kernel.

Strategy: token-shard across 8 NeuronCores. Host routes each token to one of
6 vocab units (clusters 1 and 2 are split in half so local row indices fit
int16 for dma_gather), pads each unit's token list so every core gets an
identical count, and stages:
  - bf16 tables (narrow tables padded to 128 cols so gather rows are >=256B)
  - bf16 pre-transposed projections (scaled by sqrt(D_PROJ)), chunk-major
  - per-core int16 gather index tensors (16-partition wrapped, replicated);
    pad slots hold -1 so the gather ucode skips their descriptors
Device (per core), raw bass with explicit semaphores:
  Pool : load gather ucode library, then 6 dma_gathers (transposed layout)
  SP   : projection chunk loads
  ACT  : index load first, then per-block output stores
  PE   : per 128-token block, contract d against projection chunks in PSUM
  DVE  : PSUM -> SBUF copies (cast bf16), one per 512-col half
Host scatters the per-core output rows back to original token positions.
"""

import math
from contextlib import ExitStack

import numpy as np
import ml_dtypes

import concourse.bacc as bacc
import concourse.mybir as mybir
from concourse.bass_utils import run_bass_kernel_spmd
from concourse.library_config import mlp as _mlp_lib

N_CORES = 8
D_PROJ = 1024
EMB_SCALE = float(D_PROJ) ** 0.5

# units: (token_left, token_right, table_name, row_lo, row_hi, d, proj_idx)
UNITS = [
    (0, 20000, "emb0", 0, 20000, 1024, 0),
    (20000, 40000, "emb1", 0, 20000, 256, 1),
    (40000, 60000, "emb1", 20000, 40000, 256, 1),
    (60000, 80000, "emb2", 0, 20000, 64, 2),
    (80000, 100000, "emb2", 20000, 40000, 64, 2),
    (100000, 128000, "emb3", 0, 28000, 16, 3),
]
PJ_DIMS = [1024, 256, 64, 16]

BF16 = ml_dtypes.bfloat16
NPS = 4  # rotating PSUM tiles ([128,1024] f32 = 2 banks each)

# Module-level handle for test harness inspection (exec_time_ns etc).
LAST_RESULT = None


def _elem_size(d):
    """Gathered row length in bf16 elements (>=256B granularity)."""
    return max(d, 128)


def _route(flat):
    """Token routing: per unit, per-core counts (equal across cores),
    gather caps, token positions and local table indices."""
    rt = []
    for (l, r, tname, rlo, rhi, d, pj) in UNITS:
        sel = (flat >= l) & (flat < r)
        pos = np.nonzero(sel)[0]
        loc = (flat[pos] - l).astype(np.int64)
        n = int(math.ceil(len(pos) / N_CORES)) if len(pos) else 0
        pad = n * N_CORES - len(pos)
        pos_p = np.concatenate([pos, np.full(pad, -1, np.int64)])
        loc_p = np.concatenate([loc, np.zeros(pad, np.int64)])
        rt.append({
            "n": n,
            "cap": ((n + 127) // 128) * 128,
            "nblocks": (n + 127) // 128,
            "pos": pos_p.reshape(N_CORES, n) if n else None,
            "loc": loc_p.reshape(N_CORES, n) if n else None,
            "d": d, "pj": pj,
        })
    return rt


def _build(rt):
    nc = bacc.Bacc("TRN2", target_bir_lowering=False, num_devices=N_CORES)
    act = [u for u in range(len(UNITS)) if rt[u]["n"] > 0]

    tab_dram = {u: nc.dram_tensor(
        f"t{u}", [UNITS[u][4] - UNITS[u][3], _elem_size(rt[u]["d"])],
        mybir.dt.bfloat16, kind="ExternalInput") for u in act}
    idx_cols = sum(rt[u]["cap"] // 16 for u in act)
    idx_all = nc.dram_tensor("idx", [128, idx_cols], mybir.dt.int16,
                             kind="ExternalInput")
    used_pj = sorted({rt[u]["pj"] for u in act})
    pjts = {pj: nc.dram_tensor(f"p{pj}",
                               [min(PJ_DIMS[pj], 128),
                                max(PJ_DIMS[pj] // 128, 1) * D_PROJ],
                               mybir.dt.bfloat16, kind="ExternalInput")
            for pj in used_pj}
    out = nc.dram_tensor("out", [sum(rt[u]["cap"] for u in act), D_PROJ],
                         mybir.dt.bfloat16, kind="ExternalOutput")

    stack = ExitStack()
    sb = lambda name, shape, dt: stack.enter_context(
        nc.sbuf_tensor(name, list(shape), dt))
    pt_ = lambda name, shape, dt: stack.enter_context(
        nc.psum_tensor(name, list(shape), dt))
    sem = lambda name: stack.enter_context(nc.semaphore(name))

    with stack:
        it_all = sb("idxs", [128, idx_cols], mybir.dt.int16)
        idx_off, o = {}, 0
        for u in act:
            idx_off[u] = o
            o += rt[u]["cap"] // 16
        et_t = {u: sb(f"et{u}",
                      [128, _elem_size(rt[u]["d"]) // 128, rt[u]["cap"]],
                      mybir.dt.bfloat16) for u in act}
        pjt_t = {}
        n_pj_dma = 0
        for pj in used_pj:
            d = PJ_DIMS[pj]
            part, nchunk = min(d, 128), max(d // 128, 1)
            tiles = []
            for c0 in range(0, nchunk, 2):
                w = min(2, nchunk - c0)
                t = sb(f"pjt{pj}_{c0}", [part, w * D_PROJ],
                       mybir.dt.bfloat16)
                n_pj_dma += 1
                for i in range(w):
                    tiles.append((t, i, n_pj_dma))
            pjt_t[pj] = tiles
        og_t = {u: sb(f"og{u}", [128, rt[u]["nblocks"] * D_PROJ],
                      mybir.dt.bfloat16) for u in act}
        ps_t = [pt_(f"ps{i}", [128, D_PROJ], mybir.dt.float32)
                for i in range(NPS)]

        isem = sem("isem")
        psem = sem("psem")
        gsem = {u: sem(f"gsem{u}") for u in act}
        mm_sem = sem("mm_sem")
        cp_sem = sem("cp_sem")
        osem = sem("osem")

        blocks = []  # (unit, b, valid-rows)
        for u in act:
            for b in range(rt[u]["nblocks"]):
                mm = min(128, rt[u]["n"] - b * 128)
                blocks.append((u, b, mm))
        out_off, oo = {}, 0
        for u in act:
            out_off[u] = oo
            oo += rt[u]["cap"]

        with nc.Block(name="main", no_gpsimd_drain=True) as block:

            @block.gpsimd
            def _(gp):
                gp.load_library(_mlp_lib)
                gp.wait_ge(isem, 16)
                for u in act:
                    gp.dma_gather(
                        et_t[u][:], tab_dram[u][:],
                        it_all[:, idx_off[u]: idx_off[u] + rt[u]["cap"] // 16],
                        rt[u]["cap"], rt[u]["n"], _elem_size(rt[u]["d"]),
                        transpose=True,
                    ).then_inc(gsem[u], 16)

            @block.scalar
            def _(sc):
                sc.dma_start(it_all[:], idx_all[:]).then_inc(isem, 16)
                for i, (u, b, mm) in enumerate(blocks):
                    sc.wait_ge(cp_sem, 2 * (i + 1))
                    r0 = out_off[u] + b * 128
                    sc.dma_start(
                        out[r0:r0 + 128, :],
                        og_t[u][:, b * D_PROJ:(b + 1) * D_PROJ],
                    ).then_inc(osem, 16)
                sc.wait_ge(osem, 16 * len(blocks))

            @block.sync
            def _(sy):
                for pj in used_pj:
                    d = PJ_DIMS[pj]
                    nchunk = max(d // 128, 1)
                    for c0 in range(0, nchunk, 2):
                        w = min(2, nchunk - c0)
                        t = pjt_t[pj][c0][0]
                        sy.dma_start(
                            t[:], pjts[pj][:, c0 * D_PROJ:(c0 + w) * D_PROJ]
                        ).then_inc(psem, 16)

            @block.tensor
            def _(te):
                seen = set()
                for i, (u, b, mm) in enumerate(blocks):
                    pj = rt[u]["pj"]
                    kdim = min(rt[u]["d"], 128)
                    nchunk = max(rt[u]["d"] // 128, 1)
                    if u not in seen:
                        seen.add(u)
                        te.wait_ge(gsem[u], 16)
                        need = max(dma_i for _, _, dma_i in pjt_t[pj])
                        te.wait_ge(psem, 16 * need)
                    if i >= NPS:
                        te.wait_ge(cp_sem, 2 * (i - NPS + 1))
                    ps = ps_t[i % NPS]
                    for h in range(2):
                        last = None
                        for c in range(nchunk):
                            pt, ci, _ = pjt_t[pj][c]
                            last = te.matmul(
                                ps[:mm, h * 512:(h + 1) * 512],
                                et_t[u][:kdim, c, b * 128: b * 128 + mm],
                                pt[:kdim, ci * D_PROJ + h * 512:
                                   ci * D_PROJ + h * 512 + 512],
                                start=(c == 0),
                                stop=(c == nchunk - 1),
                            )
                        last.then_inc(mm_sem, 1)

            @block.vector
            def _(ve):
                for i, (u, b, mm) in enumerate(blocks):
                    for h in range(2):
                        ve.wait_ge(mm_sem, 2 * i + h + 1)
                        ve.tensor_copy(
                            og_t[u][:mm, b * D_PROJ + h * 512:
                                    b * D_PROJ + (h + 1) * 512],
                            ps_t[i % NPS][:mm, h * 512:(h + 1) * 512],
                        ).then_inc(cp_sem, 1)

        nc.compile()
    return nc


def kernel(input, emb0, emb1, emb2, emb3, proj0, proj1, proj2, proj3):
    global LAST_RESULT
    inp = np.asarray(input)
    flat = inp.reshape(-1).astype(np.int64)
    T = flat.shape[0]
    tables = {"emb0": np.asarray(emb0), "emb1": np.asarray(emb1),
              "emb2": np.asarray(emb2), "emb3": np.asarray(emb3)}
    projs = [np.asarray(proj0), np.asarray(proj1),
             np.asarray(proj2), np.asarray(proj3)]

    rt = _route(flat)
    act = [u for u in range(len(UNITS)) if rt[u]["n"] > 0]

    # --- stage tables (bf16, narrow rows padded to 128 cols) ---------------
    tab_stage = {}
    for u in act:
        l, r, tname, rlo, rhi, d, pj = UNITS[u]
        sl = tables[tname][rlo:rhi].astype(BF16)
        es = _elem_size(d)
        if es != d:
            padded = np.zeros((sl.shape[0], es), dtype=BF16)
            padded[:, :d] = sl
            sl = padded
        tab_stage[u] = np.ascontiguousarray(sl)

    # --- stage projections: (proj.T * EMB_SCALE), chunk-major bf16 ---------
    pjt_stage = [None] * 4
    for pj in range(4):
        d = projs[pj].shape[1]
        pt = (projs[pj].T.astype(np.float32) * EMB_SCALE)  # [d, D_PROJ]
        if d >= 128:
            nchunk = d // 128
            pt = pt.reshape(nchunk, 128, D_PROJ).transpose(1, 0, 2)
            pt = pt.reshape(128, nchunk * D_PROJ)
        pjt_stage[pj] = np.ascontiguousarray(pt.astype(BF16))

    # --- per-core combined index tensor (pads = -1) ------------------------
    idx_stage = []
    for k in range(N_CORES):
        parts = []
        for u in act:
            cap = rt[u]["cap"]
            full = np.full(cap, -1, np.int16)
            full[:rt[u]["n"]] = rt[u]["loc"][k].astype(np.int16)
            parts.append(np.tile(full.reshape(cap // 16, 16).T, (8, 1)))
        idx_stage.append(np.ascontiguousarray(np.concatenate(parts, axis=1)))

    # --- build + run -------------------------------------------------------
    nc = _build(rt)
    in_maps = []
    for k in range(N_CORES):
        mm = {"idx": idx_stage[k]}
        for u in act:
            mm[f"t{u}"] = tab_stage[u]
        for pj in sorted({rt[u]["pj"] for u in act}):
            mm[f"p{pj}"] = pjt_stage[pj]
        in_maps.append(mm)

    res = run_bass_kernel_spmd(nc, in_maps, core_ids=list(range(N_CORES)))
    LAST_RESULT = res

    # --- unpermute ---------------------------------------------------------
    out_full = np.zeros((T, D_PROJ), np.float32)
    for k in range(N_CORES):
        rows = res.results[k]["out"]
        off = 0
        for u in act:
            pos = rt[u]["pos"][k]
            valid = pos >= 0
            seg = rows[off: off + rt[u]["n"]]
            out_full[pos[valid]] = seg[valid].astype(np.float32)
            off += rt[u]["cap"]
    return out_full.reshape(*inp.shape, D_PROJ)
